# revision 5
# baseline (speedup 1.0000x reference)
"""DynamicSoftKMeansLoss on 8 Trainium2 NeuronCores.

Strategy (data-parallel over B, hardcoded for B=200000, D=256, K=5, C=16):
  - Host pads B to 8*25088 rows (pad labels=C so their one-hot is all-zero),
    shards rows across 8 cores, pre-transposes each shard to partition-major
    [128, 2, tiles, 128] and casts to bf16 on host (halves HBM traffic).
  - feat_normed rows are unit-norm, so |x|^2 == 1 exactly: no per-row norm
    computation; 1+|c|^2 is a host constant folded into the distance.
  - x is DMA'd in NB upfront chunk transfers into a persistent SBUF buffer
    (100KB/partition) so the 16 DMA queues stream back-to-back; compute for
    chunk b only waits on its own chunk's DMA.
  - Per 128-row tile: psd = -2*x.c via 2 matmuls (d split 128+128) into PSUM;
    dist = sqrt(psd + 1 + |c|^2) via exp(0.5*ln(.)) (keeps Ln/Exp in one ACT
    table); softmax weighted dist wd; min/second-min over the 5 centers
    gives, for every hypothetical closest center j, viol_j = relu(wd + margin
    - min_{k!=j} d_k).
  - All per-class reductions are ONE accumulating matmul per tile:
    seg[13, 16] += vals[r, 13]^T @ onehot(labels)[r, 16] with
    vals = [w*dist(5) | w*viol_j(5) | w*wd^2 | w | 1].
  - Engine-queue software pipelining: per batch, stage A = psd matmuls (PE),
    d2 add (DVE), transcendental block (ACT: ln, exp, exp); stage B = the
    min/softmax/viol DVE block. Issue order A(0) A(1) B(0) A(2) B(1) ... so
    each in-order engine queue always has ready work; batch b's seg matmuls
    are issued after psd of batch b+2 so PE never waits on the DVE chain.
  - Each core outputs its partial [13, 16]; host sums the 8 partials (the
    gather) and runs the tiny O(C*K) final stage (per-class argmin + mean)
    in numpy.
"""

import sys

sys.path.insert(0, "/opt/trn_rl_repo")

import numpy as np

import concourse.bass as bass
import concourse.bacc as bacc
import concourse.tile as tile
from concourse import mybir
from concourse.bass_utils import run_bass_kernel_spmd

F32 = mybir.dt.float32
BF16 = mybir.dt.bfloat16
ALU = mybir.AluOpType
ACTF = mybir.ActivationFunctionType
AX = mybir.AxisListType

B, D, K, C = 200000, 256, 5, 16
NCORES = 8
MARGIN = 0.5
BIG = float(2.0**40)

TILES = 196          # 196*128 = 25088 rows/core; 8*25088 = 200704 >= 200000
RPC = TILES * 128
GB = 49              # tiles per batch/chunk (196 = 4*49)
NM = 13              # vals metrics: w*dist(5) | w*viol(5) | w*wd2 | w | 1


def _b0(ap, n, axis="inner"):
    """Stride-0 broadcast of a 2D [128, G] (or [128, K]) AP to 3D."""
    pairs = [list(p) for p in ap.ap]
    if axis == "inner":
        newap = pairs + [[0, n]]
    else:  # outer: [128, K] -> [128, n, K]
        newap = [pairs[0], [0, n], pairs[1]]
    return bass.AP(tensor=ap.tensor, offset=ap.offset, ap=newap)


def _patch_act_tables():
    """Placement-only hint: hide Ln/Exp from every table except the combined
    natural_log_exp_and_others so Bacc's greedy table-load placement picks the
    one table that serves Ln and Exp together (ids stay valid)."""
    import concourse.bacc as _bacc
    from concourse.hw_specs import get_activation_tables as _orig

    def patched(arch):
        tabs = _orig(arch)
        keep = "natural_log_exp_and_others"
        if keep in tabs:
            for name, funcs in tabs.items():
                if name != keep:
                    funcs.discard(ACTF.Ln)
                    funcs.discard(ACTF.Exp)
        return tabs

    _bacc.get_activation_tables = patched


def build_nc(tiles=TILES, gb=GB, n_cores=NCORES):
    _patch_act_tables()
    nc = bacc.Bacc(None, num_devices=n_cores)
    nb = tiles // gb
    assert tiles % gb == 0

    # host-pretransposed bf16 XT layout: [dpart, dchunk, tile, row]
    x_dram = nc.declare_dram_parameter("x", [128, 2, tiles, 128], BF16,
                                       isOutput=False)
    # packed f32 constants: iota | lab | w | cnorm1
    NCST = C + 2 * tiles + K
    const_dram = nc.declare_dram_parameter("const", [128, NCST], F32,
                                           isOutput=False)
    cbf_dram = nc.declare_dram_parameter("cbf", [128, 2 * K], BF16,
                                         isOutput=False)
    out_dram = nc.declare_dram_parameter("out", [NM, C], F32, isOutput=True)

    with tile.TileContext(nc) as tc:
        with (
            tc.tile_pool(name="consts", bufs=1) as consts,
            tc.tile_pool(name="xin", bufs=1) as xin,
            tc.tile_pool(name="small", bufs=3) as small,
            tc.tile_pool(name="stat", bufs=3) as stat,
            tc.tile_pool(name="ps_d", bufs=3, space="PSUM") as psd_pool,
            tc.tile_pool(name="ps_seg", bufs=1, space="PSUM") as psseg,
        ):
            const_sb = consts.tile([128, NCST], F32)
            nc.sync.dma_start(const_sb[:], const_dram[:])
            cbf_sb = consts.tile([128, 2 * K], BF16, tag="cbf")
            nc.scalar.dma_start(cbf_sb[:], cbf_dram[:])
            o = 0
            iota_sb = const_sb[:, o:o + C]; o += C
            lab_sb = const_sb[:, o:o + tiles]; o += tiles
            w_sb = const_sb[:, o:o + tiles]; o += tiles
            cnorm_sb = const_sb[:, o:o + K]; o += K

            # all x chunks: issued upfront, persistent SBUF residency
            xts = []
            for b in range(nb):
                xt = xin.tile([128, 2, gb, 128], BF16, tag=f"x{b}")
                nc.gpsimd.dma_start(xt[:], x_dram[:, :, b * gb:(b + 1) * gb, :])
                xts.append(xt)

            psum_seg = psseg.tile([NM, C], F32)

            st = {}  # per-batch stage state

            def stage_a(b):
                xb = xts[b]
                psd = psd_pool.tile([128, gb, K], F32)
                for g in range(gb):
                    nc.tensor.matmul(
                        psd[:, g, :], xb[:, 0, g, :], cbf_sb[:, 0:K],
                        start=True, stop=False,
                    )
                    nc.tensor.matmul(
                        psd[:, g, :], xb[:, 1, g, :], cbf_sb[:, K:2 * K],
                        start=False, stop=True,
                    )
                vals = small.tile([128, gb, NM], F32, tag="vals")
                # d2 = psum + (1 + |c|^2)
                t_d2 = small.tile([128, gb, K], F32, tag="t_d2")
                nc.vector.tensor_tensor(
                    t_d2[:], psd[:], _b0(cnorm_sb, gb, "outer"), ALU.add,
                )
                # ACT block: dist = exp(0.5*ln(d2)); eu = exp(-dist)
                lnt = small.tile([128, gb, K], F32, tag="lnt")
                nc.scalar.activation(lnt[:], t_d2[:], ACTF.Ln)
                nc.scalar.activation(vals[:, :, 0:K], lnt[:], ACTF.Exp,
                                     scale=0.5)
                eu = small.tile([128, gb, K], F32, tag="eu")
                nc.scalar.activation(eu[:], vals[:, :, 0:K], ACTF.Exp,
                                     scale=-1.0)
                st[b] = (vals, eu)

            def seg_mm(b):
                vals, oh = st.pop(b)
                for g in range(gb):
                    t = b * gb + g
                    nc.tensor.matmul(
                        psum_seg[:], vals[:, g, :], oh[:, g, :],
                        start=(t == 0), stop=(t == tiles - 1),
                    )

            def stage_b(b):
                vals, eu = st[b]
                dist = vals[:, :, 0:K]
                m1 = stat.tile([128, gb], F32, tag="m1")
                nc.vector.tensor_reduce(m1[:], dist, axis=AX.X, op=ALU.min)
                maskB = small.tile([128, gb, K], F32, tag="maskB")
                nc.vector.tensor_tensor(maskB[:], dist, _b0(m1[:], K),
                                        ALU.is_equal)
                dmask = small.tile([128, gb, K], F32, tag="dmask")
                nc.vector.tensor_scalar(dmask[:], maskB[:], BIG, None,
                                        ALU.mult)
                nc.vector.tensor_tensor(dmask[:], dmask[:], dist, ALU.add)
                m2 = stat.tile([128, gb], F32, tag="m2")
                nc.vector.tensor_reduce(m2[:], dmask[:], axis=AX.X, op=ALU.min)
                delta = stat.tile([128, gb], F32, tag="delta")
                nc.vector.tensor_tensor(delta[:], m2[:], m1[:], ALU.subtract)
                # mo_j = min_{k!=j} d_k = m1 + (m2-m1)*[d_j==m1]
                mo = small.tile([128, gb, K], F32, tag="mo")
                nc.vector.tensor_tensor(mo[:], maskB[:], _b0(delta[:], K),
                                        ALU.mult)
                nc.vector.tensor_tensor(mo[:], mo[:], _b0(m1[:], K), ALU.add)
                # softmax-weighted dist: wd = sum(eu*d)/sum(eu)
                s = stat.tile([128, gb], F32, tag="s")
                nc.vector.tensor_reduce(s[:], eu[:], axis=AX.X, op=ALU.add)
                prod = small.tile([128, gb, K], F32, tag="prod")
                nc.vector.tensor_tensor(prod[:], eu[:], dist, ALU.mult)
                spd = stat.tile([128, gb], F32, tag="spd")
                nc.vector.tensor_reduce(spd[:], prod[:], axis=AX.X, op=ALU.add)
                rs = stat.tile([128, gb], F32, tag="rs")
                nc.vector.reciprocal(rs[:], s[:])
                wd = stat.tile([128, gb], F32, tag="wd")
                nc.vector.tensor_tensor(wd[:], spd[:], rs[:], ALU.mult)
                # vals[:, :, 10] = wd^2 ; vals[:, :, 11:13] = 1
                wd3 = wd[:].rearrange("p (g o) -> p g o", o=1)
                nc.vector.tensor_tensor(vals[:, :, 10:11], wd3, wd3, ALU.mult)
                nc.vector.memset(vals[:, :, 11:13], 1.0)
                # vals[:, :, 5:10] = viol_j = relu(wd + margin - mo_j)
                hng = small.tile([128, gb, K], F32, tag="hng")
                nc.vector.tensor_tensor(hng[:], _b0(wd[:], K), mo[:],
                                        ALU.subtract)
                nc.vector.tensor_scalar(vals[:, :, K:2 * K], hng[:], MARGIN,
                                        0.0, ALU.add, ALU.max)
                # weight cols 0..11 by w in one broadcasted op
                w_g = w_sb[:, b * gb:(b + 1) * gb]
                nc.vector.tensor_tensor(
                    vals[:, :, 0:12], vals[:, :, 0:12], _b0(w_g, 12), ALU.mult,
                )
                # one-hot labels for the segment matmul (overwrites eu's slot
                # usage pattern: oh tile allocated here, used by seg_mm later)
                oh = small.tile([128, gb, C], F32, tag="oh")
                lab_g = lab_sb[:, b * gb:(b + 1) * gb]
                nc.vector.tensor_tensor(
                    oh[:], _b0(iota_sb, gb, "outer"),
                    _b0(lab_g, C, "inner"), ALU.is_equal,
                )
                st[b] = (vals, oh)

            # software-pipelined issue order
            stage_a(0)
            stage_a(1)
            stage_b(0)
            for b in range(2, nb):
                stage_a(b)
                stage_b(b - 1)
                seg_mm(b - 2)
            stage_b(nb - 1)
            seg_mm(nb - 2)
            seg_mm(nb - 1)

            seg_sb = consts.tile([NM, C], F32, tag="seg_sb")
            nc.vector.tensor_copy(seg_sb[:], psum_seg[:])
            nc.sync.dma_start(out_dram[:], seg_sb[:])

    nc.compile()
    return nc


def _host_prep(feat, labels, label2, centers, tiles=TILES, gb=GB,
               n_cores=NCORES):
    """Pad + shard + pre-transpose + bf16-cast to per-core arrays."""
    import ml_dtypes

    rpc = tiles * 128
    bpad = rpc * n_cores
    b = feat.shape[0]

    feat = np.asarray(feat, dtype=np.float32)
    labels = np.asarray(labels)
    label2 = np.asarray(label2)
    centers = np.asarray(centers, dtype=np.float32)

    lab_f = np.full(bpad, float(C), dtype=np.float32)
    lab_f[:b] = labels.astype(np.float32)
    w_f = np.zeros(bpad, dtype=np.float32)
    w_f[:b] = (label2 == 1).astype(np.float32)
    xpad = np.zeros((bpad, D), dtype=np.float32)
    xpad[:b] = feat

    # constants
    ctilT = (-2.0 * centers.T).astype(np.float32)          # [256, 5]
    cbf = np.ascontiguousarray(
        np.concatenate([ctilT[0:128], ctilT[128:256]], axis=1)
    ).astype(ml_dtypes.bfloat16)                           # [128, 10]
    cnorm1 = 1.0 + (centers * centers).sum(axis=1).astype(np.float32)  # [5]
    iota = np.tile(np.arange(C, dtype=np.float32)[None, :], (128, 1))
    cn_rep = np.tile(cnorm1[None, :], (128, 1))

    in_maps = []
    for i in range(n_cores):
        sl = slice(i * rpc, (i + 1) * rpc)
        # XT layout [dpart, dchunk, tile, row]:
        #   x[dp, c, t, r] = feat[t*128 + r, c*128 + dp]
        xi = np.ascontiguousarray(
            xpad[sl].reshape(tiles, 128, 2, 128).transpose(3, 2, 0, 1)
        ).astype(ml_dtypes.bfloat16)
        li = np.ascontiguousarray(lab_f[sl].reshape(tiles, 128).T)
        wi = np.ascontiguousarray(w_f[sl].reshape(tiles, 128).T)
        const = np.concatenate([iota, li, wi, cn_rep], axis=1)
        in_maps.append(
            {"x": xi, "const": np.ascontiguousarray(const), "cbf": cbf}
        )
    return in_maps


def _host_final(seg):
    """Final stage on the all-reduced [13, 16] stats (exact reference math)."""
    seg = seg.astype(np.float64)
    sum_dist = seg[0:K].T          # [C, K]
    sum_violj = seg[K:2 * K].T     # [C, K]
    sum_wd2 = seg[10]              # [C]
    cnt = seg[11]                  # [C]
    present = seg[12]              # [C]
    safe = np.maximum(cnt, 1.0)
    closest = np.argmin(sum_dist / safe[:, None], axis=1)
    sum_viol = sum_violj[np.arange(C), closest]
    has = (cnt > 0).astype(np.float64)
    per_class = (sum_wd2 + sum_viol) / safe * has
    n_unique = max(float((present > 0).sum()), 1.0)
    return np.float32(per_class.sum() / n_unique)


_NC_CACHE = {}


def kernel(feat_normed, labels, label2, num_classes, centers, _trace=False):
    if "nc" not in _NC_CACHE:
        _NC_CACHE["nc"] = build_nc()
    nc = _NC_CACHE["nc"]
    in_maps = _host_prep(feat_normed, labels, label2, centers)
    res = run_bass_kernel_spmd(
        nc, in_maps, core_ids=list(range(NCORES)), trace=_trace
    )
    seg = np.zeros((NM, C), dtype=np.float64)
    for r in res.results:
        seg += np.asarray(r["out"], dtype=np.float64)
    if _trace:
        kernel.last_result = res
    return np.asarray(_host_final(seg), dtype=np.float32)


# revision 8
# speedup vs baseline: 1.1575x; 1.1575x over previous
"""DynamicSoftKMeansLoss on 8 Trainium2 NeuronCores.

Strategy (data-parallel over B, hardcoded for B=200000, D=256, K=5, C=16):
  - Host pads B to 8*25088 rows (pad labels=C so their one-hot is all-zero),
    shards rows across 8 cores, pre-transposes each shard to partition-major
    [128, 2, tiles, 128] and casts to bf16 on host (halves HBM traffic).
  - feat_normed rows are unit-norm, so |x|^2 == 1 exactly: no per-row norm
    computation; 1+|c|^2 is a host constant folded into the distance.
  - x is DMA'd in NB upfront chunk transfers into a persistent SBUF buffer
    (100KB/partition) so the 16 DMA queues stream back-to-back; compute for
    chunk b only waits on its own chunk's DMA.
  - Per 128-row tile: psd = -2*x.c via 2 matmuls (d split 128+128) into PSUM;
    dist = sqrt(psd + 1 + |c|^2) via exp(0.5*ln(.)) (keeps Ln/Exp in one ACT
    table); softmax weighted dist wd; min/second-min over the 5 centers
    gives, for every hypothetical closest center j, viol_j = relu(wd + margin
    - min_{k!=j} d_k).
  - All per-class reductions are ONE accumulating matmul per tile:
    seg[13, 16] += vals[r, 13]^T @ onehot(labels)[r, 16] with
    vals = [w*dist(5) | w*viol_j(5) | w*wd^2 | w | 1].
  - Engine-queue software pipelining: per batch, stage A = psd matmuls (PE),
    d2 add (DVE), transcendental block (ACT: ln, exp, exp); stage B = the
    min/softmax/viol DVE block. Issue order A(0) A(1) B(0) A(2) B(1) ... so
    each in-order engine queue always has ready work; batch b's seg matmuls
    are issued after psd of batch b+2 so PE never waits on the DVE chain.
  - Each core outputs its partial [13, 16]; host sums the 8 partials (the
    gather) and runs the tiny O(C*K) final stage (per-class argmin + mean)
    in numpy.
"""

import sys

sys.path.insert(0, "/opt/trn_rl_repo")

import numpy as np

import concourse.bass as bass
import concourse.bacc as bacc
import concourse.tile as tile
from concourse import mybir
from concourse.bass_utils import run_bass_kernel_spmd

F32 = mybir.dt.float32
BF16 = mybir.dt.bfloat16
ALU = mybir.AluOpType
ACTF = mybir.ActivationFunctionType
AX = mybir.AxisListType

B, D, K, C = 200000, 256, 5, 16
NCORES = 8
MARGIN = 0.5
BIG = float(2.0**40)

TILES = 196          # 196*128 = 25088 rows/core; 8*25088 = 200704 >= 200000
RPC = TILES * 128
CHUNK = 14           # tiles per DMA chunk (14 chunks)
BATCHES = [28, 56, 56, 56]   # tiles per DVE batch; small first => early start
SEG_FLOOR_MS = 0.045  # sim-time floor for seg matmuls (past the x stream)
NM = 13              # vals metrics: w*dist(5) | w*viol(5) | w*wd2 | w | 1


def _b0(ap, n, axis="inner"):
    """Stride-0 broadcast of a 2D [128, G] (or [128, K]) AP to 3D."""
    pairs = [list(p) for p in ap.ap]
    if axis == "inner":
        newap = pairs + [[0, n]]
    else:  # outer: [128, K] -> [128, n, K]
        newap = [pairs[0], [0, n], pairs[1]]
    return bass.AP(tensor=ap.tensor, offset=ap.offset, ap=newap)


def _patch_act_tables():
    """Placement-only hint: hide Ln/Exp from every table except the combined
    natural_log_exp_and_others so Bacc's greedy table-load placement picks the
    one table that serves Ln and Exp together (ids stay valid)."""
    import concourse.bacc as _bacc
    from concourse.hw_specs import get_activation_tables as _orig

    def patched(arch):
        tabs = _orig(arch)
        keep = "natural_log_exp_and_others"
        if keep in tabs:
            for name, funcs in tabs.items():
                if name != keep:
                    funcs.discard(ACTF.Ln)
                    funcs.discard(ACTF.Exp)
        return tabs

    _bacc.get_activation_tables = patched


def build_nc(tiles=TILES, n_cores=NCORES):
    _patch_act_tables()
    nc = bacc.Bacc(None, num_devices=n_cores)
    batches = BATCHES
    assert sum(batches) == tiles
    nchunk = tiles // CHUNK
    starts = [sum(batches[:i]) for i in range(len(batches))]

    # host-pretransposed bf16 XT layout: [dpart, dchunk, tile, row]
    x_dram = nc.declare_dram_parameter("x", [128, 2, tiles, 128], BF16,
                                       isOutput=False)
    # packed f32 constants: iota | lab | w | cnorm1
    NCST = C + 2 * tiles + K
    const_dram = nc.declare_dram_parameter("const", [128, NCST], F32,
                                           isOutput=False)
    cbf_dram = nc.declare_dram_parameter("cbf", [128, 2 * K], BF16,
                                         isOutput=False)
    out_dram = nc.declare_dram_parameter("out", [NM, C], F32, isOutput=True)

    with tile.TileContext(nc) as tc:
        with (
            tc.tile_pool(name="consts", bufs=1) as consts,
            tc.tile_pool(name="xin", bufs=1) as xin,
            tc.tile_pool(name="big", bufs=1) as bigp,
            tc.tile_pool(name="small", bufs=3) as small,
            tc.tile_pool(name="stat", bufs=3) as stat,
            tc.tile_pool(name="ps_d", bufs=1, space="PSUM") as psd_pool,
            tc.tile_pool(name="ps_seg", bufs=1, space="PSUM") as psseg,
        ):
            const_sb = consts.tile([128, NCST], F32)
            nc.sync.dma_start(const_sb[:], const_dram[:])
            cbf_sb = consts.tile([128, 2 * K], BF16, tag="cbf")
            nc.scalar.dma_start(cbf_sb[:], cbf_dram[:])
            o = 0
            iota_sb = const_sb[:, o:o + C]; o += C
            lab_sb = const_sb[:, o:o + tiles]; o += tiles
            w_sb = const_sb[:, o:o + tiles]; o += tiles
            cnorm_sb = const_sb[:, o:o + K]; o += K

            # x chunks: fine-grained so psd matmuls start early and the
            # scheduler's sim sees a steady trickle of ready PE work
            xts = []
            for c in range(nchunk):
                xt = xin.tile([128, 2, CHUNK, 128], BF16, tag=f"x{c}")
                nc.gpsimd.dma_start(
                    xt[:], x_dram[:, :, c * CHUNK:(c + 1) * CHUNK, :]
                )
                xts.append(xt)

            psum_seg = psseg.tile([NM, C], F32)

            # one-hot + vals-const-cols depend only on consts: all upfront,
            # scheduler runs them during the DMA fill
            ohs, valss = [], []
            for b, gb in enumerate(batches):
                t0 = starts[b]
                oh = bigp.tile([128, gb, C], F32, tag=f"oh{b}")
                nc.vector.tensor_tensor(
                    oh[:], _b0(iota_sb, gb, "outer"),
                    _b0(lab_sb[:, t0:t0 + gb], C, "inner"), ALU.is_equal,
                )
                vals = bigp.tile([128, gb, NM], F32, tag=f"vals{b}")
                nc.vector.memset(vals[:, :, 11:13], 1.0)
                ohs.append(oh)
                valss.append(vals)

            def stage_a(b):
                gb = batches[b]
                t0 = starts[b]
                psd = psd_pool.tile([128, gb, K], F32, tag=f"psd{b}")
                for g in range(gb):
                    t = t0 + g
                    xt = xts[t // CHUNK]
                    r = t % CHUNK
                    nc.tensor.matmul(
                        psd[:, g, :], xt[:, 0, r, :], cbf_sb[:, 0:K],
                        start=True, stop=False,
                    )
                    nc.tensor.matmul(
                        psd[:, g, :], xt[:, 1, r, :], cbf_sb[:, K:2 * K],
                        start=False, stop=True,
                    )
                vals = valss[b]
                # d2 = psum + (1 + |c|^2)
                t_d2 = small.tile([128, gb, K], F32, tag="t_d2")
                nc.vector.tensor_tensor(
                    t_d2[:], psd[:], _b0(cnorm_sb, gb, "outer"), ALU.add,
                )
                # ACT block: dist = exp(0.5*ln(d2)); eu = exp(-dist)
                lnt = small.tile([128, gb, K], F32, tag="lnt")
                nc.scalar.activation(lnt[:], t_d2[:], ACTF.Ln)
                nc.scalar.activation(vals[:, :, 0:K], lnt[:], ACTF.Exp,
                                     scale=0.5)
                eu = small.tile([128, gb, K], F32, tag="eu")
                nc.scalar.activation(eu[:], vals[:, :, 0:K], ACTF.Exp,
                                     scale=-1.0)
                return eu

            def stage_b(b, eu):
                gb = batches[b]
                t0 = starts[b]
                vals = valss[b]
                dist = vals[:, :, 0:K]
                m1 = stat.tile([128, gb], F32, tag="m1")
                nc.vector.tensor_reduce(m1[:], dist, axis=AX.X, op=ALU.min)
                maskB = small.tile([128, gb, K], F32, tag="maskB")
                nc.vector.tensor_tensor(maskB[:], dist, _b0(m1[:], K),
                                        ALU.is_equal)
                dmask = small.tile([128, gb, K], F32, tag="dmask")
                nc.vector.tensor_scalar(dmask[:], maskB[:], BIG, None,
                                        ALU.mult)
                nc.vector.tensor_tensor(dmask[:], dmask[:], dist, ALU.add)
                m2 = stat.tile([128, gb], F32, tag="m2")
                nc.vector.tensor_reduce(m2[:], dmask[:], axis=AX.X, op=ALU.min)
                delta = stat.tile([128, gb], F32, tag="delta")
                nc.vector.tensor_tensor(delta[:], m2[:], m1[:], ALU.subtract)
                # mo_j = min_{k!=j} d_k = m1 + (m2-m1)*[d_j==m1]
                mo = small.tile([128, gb, K], F32, tag="mo")
                nc.vector.tensor_tensor(mo[:], maskB[:], _b0(delta[:], K),
                                        ALU.mult)
                nc.vector.tensor_tensor(mo[:], mo[:], _b0(m1[:], K), ALU.add)
                # softmax-weighted dist: wd = sum(eu*d)/sum(eu)
                s = stat.tile([128, gb], F32, tag="s")
                nc.vector.tensor_reduce(s[:], eu[:], axis=AX.X, op=ALU.add)
                prod = small.tile([128, gb, K], F32, tag="prod")
                nc.vector.tensor_tensor(prod[:], eu[:], dist, ALU.mult)
                spd = stat.tile([128, gb], F32, tag="spd")
                nc.vector.tensor_reduce(spd[:], prod[:], axis=AX.X, op=ALU.add)
                rs = stat.tile([128, gb], F32, tag="rs")
                nc.vector.reciprocal(rs[:], s[:])
                wd = stat.tile([128, gb], F32, tag="wd")
                nc.vector.tensor_tensor(wd[:], spd[:], rs[:], ALU.mult)
                # vals[:, :, 10] = wd^2 (cols 11:13 pre-set to 1)
                wd3 = wd[:].rearrange("p (g o) -> p g o", o=1)
                nc.vector.tensor_tensor(vals[:, :, 10:11], wd3, wd3, ALU.mult)
                # vals[:, :, 5:10] = viol_j = relu(wd + margin - mo_j)
                hng = small.tile([128, gb, K], F32, tag="hng")
                nc.vector.tensor_tensor(hng[:], _b0(wd[:], K), mo[:],
                                        ALU.subtract)
                nc.vector.tensor_scalar(vals[:, :, K:2 * K], hng[:], MARGIN,
                                        0.0, ALU.add, ALU.max)
                # weight cols 0..11 by w in one broadcasted op
                w_g = w_sb[:, t0:t0 + gb]
                nc.vector.tensor_tensor(
                    vals[:, :, 0:12], vals[:, :, 0:12], _b0(w_g, 12), ALU.mult,
                )

            for b in range(len(batches)):
                eu = stage_a(b)
                stage_b(b, eu)

            # seg matmuls: sim-time floor keeps them from being scheduled
            # between psd matmul groups (which would serialize PE behind the
            # DVE chains); accumulation-chain order is emission order
            with tc.tile_wait_until(SEG_FLOOR_MS):
                first = True
                for b, gb in enumerate(batches):
                    vals, oh = valss[b], ohs[b]
                    for g in range(gb):
                        t = starts[b] + g
                        nc.tensor.matmul(
                            psum_seg[:], vals[:, g, :], oh[:, g, :],
                            start=first, stop=(t == tiles - 1),
                        )
                        first = False

            seg_sb = consts.tile([NM, C], F32, tag="seg_sb")
            nc.vector.tensor_copy(seg_sb[:], psum_seg[:])
            nc.sync.dma_start(out_dram[:], seg_sb[:])

    nc.compile()
    return nc


def _host_prep(feat, labels, label2, centers, tiles=TILES,
               n_cores=NCORES):
    """Pad + shard + pre-transpose + bf16-cast to per-core arrays."""
    import ml_dtypes

    rpc = tiles * 128
    bpad = rpc * n_cores
    b = feat.shape[0]

    feat = np.asarray(feat, dtype=np.float32)
    labels = np.asarray(labels)
    label2 = np.asarray(label2)
    centers = np.asarray(centers, dtype=np.float32)

    lab_f = np.full(bpad, float(C), dtype=np.float32)
    lab_f[:b] = labels.astype(np.float32)
    w_f = np.zeros(bpad, dtype=np.float32)
    w_f[:b] = (label2 == 1).astype(np.float32)
    xpad = np.zeros((bpad, D), dtype=np.float32)
    xpad[:b] = feat

    # constants
    ctilT = (-2.0 * centers.T).astype(np.float32)          # [256, 5]
    cbf = np.ascontiguousarray(
        np.concatenate([ctilT[0:128], ctilT[128:256]], axis=1)
    ).astype(ml_dtypes.bfloat16)                           # [128, 10]
    cnorm1 = 1.0 + (centers * centers).sum(axis=1).astype(np.float32)  # [5]
    iota = np.tile(np.arange(C, dtype=np.float32)[None, :], (128, 1))
    cn_rep = np.tile(cnorm1[None, :], (128, 1))

    in_maps = []
    for i in range(n_cores):
        sl = slice(i * rpc, (i + 1) * rpc)
        # XT layout [dpart, dchunk, tile, row]:
        #   x[dp, c, t, r] = feat[t*128 + r, c*128 + dp]
        xi = np.ascontiguousarray(
            xpad[sl].reshape(tiles, 128, 2, 128).transpose(3, 2, 0, 1)
        ).astype(ml_dtypes.bfloat16)
        li = np.ascontiguousarray(lab_f[sl].reshape(tiles, 128).T)
        wi = np.ascontiguousarray(w_f[sl].reshape(tiles, 128).T)
        const = np.concatenate([iota, li, wi, cn_rep], axis=1)
        in_maps.append(
            {"x": xi, "const": np.ascontiguousarray(const), "cbf": cbf}
        )
    return in_maps


def _host_final(seg):
    """Final stage on the all-reduced [13, 16] stats (exact reference math)."""
    seg = seg.astype(np.float64)
    sum_dist = seg[0:K].T          # [C, K]
    sum_violj = seg[K:2 * K].T     # [C, K]
    sum_wd2 = seg[10]              # [C]
    cnt = seg[11]                  # [C]
    present = seg[12]              # [C]
    safe = np.maximum(cnt, 1.0)
    closest = np.argmin(sum_dist / safe[:, None], axis=1)
    sum_viol = sum_violj[np.arange(C), closest]
    has = (cnt > 0).astype(np.float64)
    per_class = (sum_wd2 + sum_viol) / safe * has
    n_unique = max(float((present > 0).sum()), 1.0)
    return np.float32(per_class.sum() / n_unique)


_NC_CACHE = {}


def kernel(feat_normed, labels, label2, num_classes, centers, _trace=False):
    if "nc" not in _NC_CACHE:
        _NC_CACHE["nc"] = build_nc()
    nc = _NC_CACHE["nc"]
    in_maps = _host_prep(feat_normed, labels, label2, centers)
    res = run_bass_kernel_spmd(
        nc, in_maps, core_ids=list(range(NCORES)), trace=_trace
    )
    seg = np.zeros((NM, C), dtype=np.float64)
    for r in res.results:
        seg += np.asarray(r["out"], dtype=np.float64)
    if _trace:
        kernel.last_result = res
    return np.asarray(_host_final(seg), dtype=np.float32)


# revision 10
# speedup vs baseline: 1.1615x; 1.0035x over previous
"""DynamicSoftKMeansLoss on 8 Trainium2 NeuronCores.

Strategy (data-parallel over B, hardcoded for B=200000, D=256, K=5, C=16):
  - Host pads B to 8*25088 rows (pad labels=C so their one-hot is all-zero),
    shards rows across 8 cores, pre-transposes each shard to partition-major
    [128, 2, tiles, 128] and casts to bf16 on host (halves HBM traffic).
  - feat_normed rows are unit-norm, so |x|^2 == 1 exactly; 1+|c|^2 is a host
    constant folded into the distance.
  - x is DMA'd in 14 fine-grained chunks issued upfront from two sequencers
    (gpsimd + sync) into persistent SBUF (100KB/partition): the 16 DMA queues
    stream back-to-back and psd matmuls trickle in behind each chunk.
  - Per 128-row tile: psd = -2*x.c via 2 matmuls (d split 128+128) into PSUM;
    dist = sqrt(psd + 1 + |c|^2) via exp(0.5*ln(.)) on ACT; softmax weighted
    dist wd; min/second-min over the 5 centers gives, for every hypothetical
    closest center j, viol_j = relu(wd + margin - min_{k!=j} d_k).
  - Intermediates are bf16 (2x DVE on packed ops); per-class sums stay exact
    in f32 PSUM.
  - The label2 gate w is folded into the one-hot instead of vals: the segment
    matmul is seg[12, 32] += vals[r, 12]^T @ [w*onehot | onehot][r, 32] with
    vals = [dist(5) | viol_j(5) | wd^2 | 1]; both one-hot blocks depend only
    on constants and are computed during the DMA fill.
  - Work is split into 5 batches [14, 42, 56, 56, 28]: small first batch
    starts the DVE chain early, small last batch keeps the post-stream tail
    short. Two seg PSUM banks (batches 0-3 / batch 4) so the final seg chain
    is 28 matmuls, not 196; seg bank A is floored past the x stream in the
    scheduler's sim so it never blocks PE between psd groups.
  - Each core outputs its partial [12, 32]; host sums the 8 partials (the
    gather) and runs the tiny O(C*K) final stage (per-class argmin + mean)
    in numpy.
"""

import sys

sys.path.insert(0, "/opt/trn_rl_repo")

import numpy as np

import concourse.bass as bass
import concourse.bacc as bacc
import concourse.tile as tile
from concourse import mybir
from concourse.bass_utils import run_bass_kernel_spmd

F32 = mybir.dt.float32
BF16 = mybir.dt.bfloat16
ALU = mybir.AluOpType
ACTF = mybir.ActivationFunctionType
AX = mybir.AxisListType

B, D, K, C = 200000, 256, 5, 16
NCORES = 8
MARGIN = 0.5
BIG = float(2.0**10)

TILES = 196          # 196*128 = 25088 rows/core; 8*25088 = 200704 >= 200000
RPC = TILES * 128
CHUNK = 14           # tiles per DMA chunk (14 chunks)
BATCHES = [14, 42, 56, 56, 28]  # small first => early start; small last => short tail
SEG_FLOOR_MS = 0.045  # sim-time floor for seg bank A (past the x stream)
NM = 12              # vals metrics: dist(5) | viol_j(5) | wd^2 | 1
OHC = 2 * C          # [w*onehot | onehot]


def _b0(ap, n, axis="inner"):
    """Stride-0 broadcast of a 2D [128, G] (or [128, K]) AP to 3D."""
    pairs = [list(p) for p in ap.ap]
    if axis == "inner":
        newap = pairs + [[0, n]]
    else:  # outer: [128, K] -> [128, n, K]
        newap = [pairs[0], [0, n], pairs[1]]
    return bass.AP(tensor=ap.tensor, offset=ap.offset, ap=newap)


def _patch_act_tables():
    """Placement-only hint: hide Ln/Exp from every table except the combined
    natural_log_exp_and_others so Bacc's greedy table-load placement picks the
    one table that serves Ln and Exp together (ids stay valid)."""
    import concourse.bacc as _bacc
    from concourse.hw_specs import get_activation_tables as _orig

    def patched(arch):
        tabs = _orig(arch)
        keep = "natural_log_exp_and_others"
        if keep in tabs:
            for name, funcs in tabs.items():
                if name != keep:
                    funcs.discard(ACTF.Ln)
                    funcs.discard(ACTF.Exp)
        return tabs

    _bacc.get_activation_tables = patched


def build_nc(tiles=TILES, n_cores=NCORES):
    _patch_act_tables()
    nc = bacc.Bacc(None, num_devices=n_cores)
    batches = BATCHES
    assert sum(batches) == tiles
    nchunk = tiles // CHUNK
    starts = [sum(batches[:i]) for i in range(len(batches))]

    # host-pretransposed bf16 XT layout: [dpart, dchunk, tile, row]
    x_dram = nc.declare_dram_parameter("x", [128, 2, tiles, 128], BF16,
                                       isOutput=False)
    # packed f32 constants: iota | lab | w | cnorm1
    NCST = C + 2 * tiles + K
    const_dram = nc.declare_dram_parameter("const", [128, NCST], F32,
                                           isOutput=False)
    cbf_dram = nc.declare_dram_parameter("cbf", [128, 2 * K], BF16,
                                         isOutput=False)
    out_dram = nc.declare_dram_parameter("out", [NM, OHC], F32, isOutput=True)

    with tile.TileContext(nc) as tc:
        with (
            tc.tile_pool(name="consts", bufs=1) as consts,
            tc.tile_pool(name="xin", bufs=1) as xin,
            tc.tile_pool(name="big", bufs=1) as bigp,
            tc.tile_pool(name="small", bufs=3) as small,
            tc.tile_pool(name="stat", bufs=3) as stat,
            tc.tile_pool(name="ps_d", bufs=1, space="PSUM") as psd_pool,
            tc.tile_pool(name="ps_seg", bufs=1, space="PSUM") as psseg,
        ):
            const_sb = consts.tile([128, NCST], F32)
            nc.sync.dma_start(const_sb[:], const_dram[:])
            cbf_sb = consts.tile([128, 2 * K], BF16, tag="cbf")
            nc.scalar.dma_start(cbf_sb[:], cbf_dram[:])
            o = 0
            iota_sb = const_sb[:, o:o + C]; o += C
            lab_sb = const_sb[:, o:o + tiles]; o += tiles
            w_sb = const_sb[:, o:o + tiles]; o += tiles
            cnorm_sb = const_sb[:, o:o + K]; o += K

            # x chunks: fine-grained, issued upfront from two sequencers
            xts = []
            for c in range(nchunk):
                xt = xin.tile([128, 2, CHUNK, 128], BF16, tag=f"x{c}")
                eng = nc.gpsimd if c % 2 == 0 else nc.sync
                eng.dma_start(
                    xt[:], x_dram[:, :, c * CHUNK:(c + 1) * CHUNK, :]
                )
                xts.append(xt)

            psum_segA = psseg.tile([NM, OHC], F32, tag="segA")
            psum_segB = psseg.tile([NM, OHC], F32, tag="segB")

            # one-hot blocks + vals const col: depend only on consts, the
            # scheduler runs them during the DMA fill
            ohs, valss = [], []
            for b, gb in enumerate(batches):
                t0 = starts[b]
                oh = bigp.tile([128, gb, OHC], BF16, tag=f"oh{b}")
                nc.vector.tensor_tensor(
                    oh[:, :, C:OHC], _b0(iota_sb, gb, "outer"),
                    _b0(lab_sb[:, t0:t0 + gb], C, "inner"), ALU.is_equal,
                )
                nc.vector.tensor_tensor(
                    oh[:, :, 0:C], oh[:, :, C:OHC],
                    _b0(w_sb[:, t0:t0 + gb], C, "inner"), ALU.mult,
                )
                vals = bigp.tile([128, gb, NM], BF16, tag=f"vals{b}")
                nc.vector.memset(vals[:, :, 11:12], 1.0)
                ohs.append(oh)
                valss.append(vals)

            def stage_a(b):
                gb = batches[b]
                t0 = starts[b]
                psd = psd_pool.tile([128, gb, K], F32, tag=f"psd{b}")
                for g in range(gb):
                    t = t0 + g
                    xt = xts[t // CHUNK]
                    r = t % CHUNK
                    nc.tensor.matmul(
                        psd[:, g, :], xt[:, 0, r, :], cbf_sb[:, 0:K],
                        start=True, stop=False,
                    )
                    nc.tensor.matmul(
                        psd[:, g, :], xt[:, 1, r, :], cbf_sb[:, K:2 * K],
                        start=False, stop=True,
                    )
                vals = valss[b]
                # d2 = psum + (1 + |c|^2)
                t_d2 = small.tile([128, gb, K], BF16, tag="t_d2")
                nc.vector.tensor_tensor(
                    t_d2[:], psd[:], _b0(cnorm_sb, gb, "outer"), ALU.add,
                )
                # ACT block: dist = exp(0.5*ln(d2)); eu = exp(-dist)
                lnt = small.tile([128, gb, K], F32, tag="lnt")
                nc.scalar.activation(lnt[:], t_d2[:], ACTF.Ln)
                nc.scalar.activation(vals[:, :, 0:K], lnt[:], ACTF.Exp,
                                     scale=0.5)
                eu = small.tile([128, gb, K], BF16, tag="eu")
                nc.scalar.activation(eu[:], vals[:, :, 0:K], ACTF.Exp,
                                     scale=-1.0)
                return eu

            def stage_b(b, eu):
                gb = batches[b]
                vals = valss[b]
                dist = vals[:, :, 0:K]
                m1 = stat.tile([128, gb], BF16, tag="m1")
                nc.vector.tensor_reduce(m1[:], dist, axis=AX.X, op=ALU.min)
                maskB = small.tile([128, gb, K], BF16, tag="maskB")
                nc.vector.tensor_tensor(maskB[:], dist, _b0(m1[:], K),
                                        ALU.is_equal)
                dmask = small.tile([128, gb, K], BF16, tag="dmask")
                nc.vector.tensor_scalar(dmask[:], maskB[:], BIG, None,
                                        ALU.mult)
                nc.vector.tensor_tensor(dmask[:], dmask[:], dist, ALU.add)
                m2 = stat.tile([128, gb], BF16, tag="m2")
                nc.vector.tensor_reduce(m2[:], dmask[:], axis=AX.X, op=ALU.min)
                delta = stat.tile([128, gb], BF16, tag="delta")
                nc.vector.tensor_tensor(delta[:], m2[:], m1[:], ALU.subtract)
                # softmax-weighted dist: wd = sum(eu*d)/sum(eu)
                s = stat.tile([128, gb], F32, tag="s")
                nc.vector.tensor_reduce(s[:], eu[:], axis=AX.X, op=ALU.add)
                prod = small.tile([128, gb, K], BF16, tag="prod")
                nc.vector.tensor_tensor(prod[:], eu[:], dist, ALU.mult)
                spd = stat.tile([128, gb], F32, tag="spd")
                nc.vector.tensor_reduce(spd[:], prod[:], axis=AX.X, op=ALU.add)
                rs = stat.tile([128, gb], F32, tag="rs")
                nc.vector.reciprocal(rs[:], s[:])
                wd = stat.tile([128, gb], F32, tag="wd")
                nc.vector.tensor_tensor(wd[:], spd[:], rs[:], ALU.mult)
                # vals[:, :, 10] = wd^2 (col 11 pre-set to 1)
                wd3 = wd[:].rearrange("p (g o) -> p g o", o=1)
                nc.vector.tensor_tensor(vals[:, :, 10:11], wd3, wd3, ALU.mult)
                # viol_j = relu(wd + margin - mo_j), mo_j = min_{k!=j} d_k
                #        = m1 + (m2-m1)*[d_j==m1]
                # hng_j = (wd - m1) - (m2-m1)*[d_j==m1]; viol = max(hng+M, 0)
                wdm1 = stat.tile([128, gb], F32, tag="wdm1")
                nc.vector.tensor_tensor(wdm1[:], wd[:], m1[:], ALU.subtract)
                hng = small.tile([128, gb, K], BF16, tag="hng")
                nc.vector.tensor_tensor(hng[:], maskB[:], _b0(delta[:], K),
                                        ALU.mult)
                nc.vector.tensor_tensor(hng[:], _b0(wdm1[:], K), hng[:],
                                        ALU.subtract)
                nc.vector.tensor_scalar(vals[:, :, K:2 * K], hng[:], MARGIN,
                                        0.0, ALU.add, ALU.max)

            for b in range(len(batches)):
                eu = stage_a(b)
                stage_b(b, eu)

            nbat = len(batches)
            # seg bank A: batches 0..nbat-2, sim-floored past the x stream so
            # the scheduler never wedges it between psd matmul groups
            with tc.tile_wait_until(SEG_FLOOR_MS):
                first = True
                for b in range(nbat - 1):
                    vals, oh, gb = valss[b], ohs[b], batches[b]
                    for g in range(gb):
                        nc.tensor.matmul(
                            psum_segA[:], vals[:, g, :], oh[:, g, :],
                            start=first,
                            stop=(b == nbat - 2 and g == gb - 1),
                        )
                        first = False
            # seg bank B: the small last batch (short post-stream tail)
            b = nbat - 1
            vals, oh, gb = valss[b], ohs[b], batches[b]
            for g in range(gb):
                nc.tensor.matmul(
                    psum_segB[:], vals[:, g, :], oh[:, g, :],
                    start=(g == 0), stop=(g == gb - 1),
                )

            segb_sb = consts.tile([NM, OHC], F32, tag="segb_sb")
            nc.vector.tensor_copy(segb_sb[:], psum_segB[:])
            seg_sb = consts.tile([NM, OHC], F32, tag="seg_sb")
            nc.vector.tensor_tensor(seg_sb[:], psum_segA[:], segb_sb[:],
                                    ALU.add)
            nc.sync.dma_start(out_dram[:], seg_sb[:])

    nc.compile()
    return nc


def _host_prep(feat, labels, label2, centers, tiles=TILES,
               n_cores=NCORES):
    """Pad + shard + pre-transpose + bf16-cast to per-core arrays."""
    import ml_dtypes

    rpc = tiles * 128
    bpad = rpc * n_cores
    b = feat.shape[0]

    feat = np.asarray(feat, dtype=np.float32)
    labels = np.asarray(labels)
    label2 = np.asarray(label2)
    centers = np.asarray(centers, dtype=np.float32)

    lab_f = np.full(bpad, float(C), dtype=np.float32)
    lab_f[:b] = labels.astype(np.float32)
    w_f = np.zeros(bpad, dtype=np.float32)
    w_f[:b] = (label2 == 1).astype(np.float32)
    xpad = np.zeros((bpad, D), dtype=np.float32)
    xpad[:b] = feat

    # constants
    ctilT = (-2.0 * centers.T).astype(np.float32)          # [256, 5]
    cbf = np.ascontiguousarray(
        np.concatenate([ctilT[0:128], ctilT[128:256]], axis=1)
    ).astype(ml_dtypes.bfloat16)                           # [128, 10]
    cnorm1 = 1.0 + (centers * centers).sum(axis=1).astype(np.float32)  # [5]
    iota = np.tile(np.arange(C, dtype=np.float32)[None, :], (128, 1))
    cn_rep = np.tile(cnorm1[None, :], (128, 1))

    in_maps = []
    for i in range(n_cores):
        sl = slice(i * rpc, (i + 1) * rpc)
        # XT layout [dpart, dchunk, tile, row]:
        #   x[dp, c, t, r] = feat[t*128 + r, c*128 + dp]
        xi = np.ascontiguousarray(
            xpad[sl].reshape(tiles, 128, 2, 128).transpose(3, 2, 0, 1)
        ).astype(ml_dtypes.bfloat16)
        li = np.ascontiguousarray(lab_f[sl].reshape(tiles, 128).T)
        wi = np.ascontiguousarray(w_f[sl].reshape(tiles, 128).T)
        const = np.concatenate([iota, li, wi, cn_rep], axis=1)
        in_maps.append(
            {"x": xi, "const": np.ascontiguousarray(const), "cbf": cbf}
        )
    return in_maps


def _host_final(seg):
    """Final stage on the all-reduced [12, 32] stats (exact reference math).
    Cols 0:16 are w-weighted sums, cols 16:32 unweighted."""
    seg = seg.astype(np.float64)
    wblk = seg[:, 0:C]
    pblk = seg[:, C:OHC]
    sum_dist = wblk[0:K].T         # [C, K]
    sum_violj = wblk[K:2 * K].T    # [C, K]
    sum_wd2 = wblk[10]             # [C]
    cnt = wblk[11]                 # [C]
    present = pblk[11]             # [C]
    safe = np.maximum(cnt, 1.0)
    closest = np.argmin(sum_dist / safe[:, None], axis=1)
    sum_viol = sum_violj[np.arange(C), closest]
    has = (cnt > 0).astype(np.float64)
    per_class = (sum_wd2 + sum_viol) / safe * has
    n_unique = max(float((present > 0).sum()), 1.0)
    return np.float32(per_class.sum() / n_unique)


_NC_CACHE = {}


def kernel(feat_normed, labels, label2, num_classes, centers, _trace=False):
    if "nc" not in _NC_CACHE:
        _NC_CACHE["nc"] = build_nc()
    nc = _NC_CACHE["nc"]
    in_maps = _host_prep(feat_normed, labels, label2, centers)
    res = run_bass_kernel_spmd(
        nc, in_maps, core_ids=list(range(NCORES)), trace=_trace
    )
    seg = np.zeros((NM, OHC), dtype=np.float64)
    for r in res.results:
        seg += np.asarray(r["out"], dtype=np.float64)
    if _trace:
        kernel.last_result = res
    return np.asarray(_host_final(seg), dtype=np.float32)


# revision 11
# speedup vs baseline: 1.5306x; 1.3178x over previous
"""DynamicSoftKMeansLoss on 8 Trainium2 NeuronCores.

Strategy (data-parallel over B, hardcoded for B=200000, D=256, K=5, C=16):
  - Host pads B to 8*25088 rows (pad labels=C so their one-hot is all-zero),
    shards rows across 8 cores, pre-transposes each shard to partition-major
    [128, 2, tiles, 128] and casts to bf16 on host (halves HBM traffic).
  - feat_normed rows are unit-norm, so |x|^2 == 1 exactly; 1+|c|^2 is a host
    constant folded into the distance.
  - x is DMA'd in 14 fine-grained chunks issued upfront from two sequencers
    (gpsimd + sync) into persistent SBUF (100KB/partition): the 16 DMA queues
    stream back-to-back and psd matmuls trickle in behind each chunk.
  - Per 128-row tile: psd = -2*x.c via 2 matmuls (d split 128+128) into PSUM;
    dist = sqrt(psd + 1 + |c|^2) via exp(0.5*ln(.)) on ACT; softmax weighted
    dist wd; min/second-min over the 5 centers gives, for every hypothetical
    closest center j, viol_j = relu(wd + margin - min_{k!=j} d_k).
  - Intermediates are bf16 (2x DVE on packed ops); per-class sums stay exact
    in f32 PSUM.
  - The label2 gate w is folded into the one-hot instead of vals: the segment
    matmul is seg[12, 32] += vals[r, 12]^T @ [w*onehot | onehot][r, 32] with
    vals = [dist(5) | viol_j(5) | wd^2 | 1]; both one-hot blocks depend only
    on constants and are computed during the DMA fill.
  - Work is split into 5 batches [14, 42, 56, 56, 28]: small first batch
    starts the DVE chain early, small last batch keeps the post-stream tail
    short. Two seg PSUM banks (batches 0-3 / batch 4) so the final seg chain
    is 28 matmuls, not 196; seg bank A is floored past the x stream in the
    scheduler's sim so it never blocks PE between psd groups.
  - Each core outputs its partial [12, 32]; host sums the 8 partials (the
    gather) and runs the tiny O(C*K) final stage (per-class argmin + mean)
    in numpy.
"""

import sys

sys.path.insert(0, "/opt/trn_rl_repo")

import numpy as np

import concourse.bass as bass
import concourse.bacc as bacc
import concourse.tile as tile
from concourse import mybir
from concourse.bass_utils import run_bass_kernel_spmd

F32 = mybir.dt.float32
BF16 = mybir.dt.bfloat16
F8 = mybir.dt.float8e4
XSCALE = 16.0
ALU = mybir.AluOpType
ACTF = mybir.ActivationFunctionType
AX = mybir.AxisListType

B, D, K, C = 200000, 256, 5, 16
NCORES = 8
MARGIN = 0.5
BIG = float(2.0**10)

TILES = 196          # 196*128 = 25088 rows/core; 8*25088 = 200704 >= 200000
RPC = TILES * 128
CHUNK = 28           # tiles per DMA chunk (7 chunks)
BATCHES = [28, 56, 56, 28, 28]  # small last => short post-stream tail
SEG_FLOOR_MS = 0.030  # sim-time floor for seg bank A (past all psd matmuls)
NM = 12              # vals metrics: dist(5) | viol_j(5) | wd^2 | 1
OHC = 2 * C          # [w*onehot | onehot]


def _b0(ap, n, axis="inner"):
    """Stride-0 broadcast of a 2D [128, G] (or [128, K]) AP to 3D."""
    pairs = [list(p) for p in ap.ap]
    if axis == "inner":
        newap = pairs + [[0, n]]
    else:  # outer: [128, K] -> [128, n, K]
        newap = [pairs[0], [0, n], pairs[1]]
    return bass.AP(tensor=ap.tensor, offset=ap.offset, ap=newap)


def _patch_act_tables():
    """Placement-only hint: hide Ln/Exp from every table except the combined
    natural_log_exp_and_others so Bacc's greedy table-load placement picks the
    one table that serves Ln and Exp together (ids stay valid)."""
    import concourse.bacc as _bacc
    from concourse.hw_specs import get_activation_tables as _orig

    def patched(arch):
        tabs = _orig(arch)
        keep = "natural_log_exp_and_others"
        if keep in tabs:
            for name, funcs in tabs.items():
                if name != keep:
                    funcs.discard(ACTF.Ln)
                    funcs.discard(ACTF.Exp)
        return tabs

    _bacc.get_activation_tables = patched


def build_nc(tiles=TILES, n_cores=NCORES):
    _patch_act_tables()
    nc = bacc.Bacc(None, num_devices=n_cores)
    batches = BATCHES
    assert sum(batches) == tiles
    nchunk = tiles // CHUNK
    starts = [sum(batches[:i]) for i in range(len(batches))]

    # host-pretransposed bf16 XT layout: [dpart, dchunk, tile, row]
    x_dram = nc.declare_dram_parameter("x", [128, 2, tiles, 128], F8,
                                       isOutput=False)
    # packed f32 constants: iota | lab | w | cnorm1
    NCST = C + 2 * tiles + K
    const_dram = nc.declare_dram_parameter("const", [128, NCST], F32,
                                           isOutput=False)
    cbf_dram = nc.declare_dram_parameter("cbf", [128, 2 * K], F8,
                                         isOutput=False)
    out_dram = nc.declare_dram_parameter("out", [NM, OHC], F32, isOutput=True)

    with tile.TileContext(nc) as tc:
        with (
            tc.tile_pool(name="consts", bufs=1) as consts,
            tc.tile_pool(name="xin", bufs=1) as xin,
            tc.tile_pool(name="big", bufs=1) as bigp,
            tc.tile_pool(name="small", bufs=3) as small,
            tc.tile_pool(name="stat", bufs=3) as stat,
            tc.tile_pool(name="ps_d", bufs=1, space="PSUM") as psd_pool,
            tc.tile_pool(name="ps_seg", bufs=1, space="PSUM") as psseg,
        ):
            const_sb = consts.tile([128, NCST], F32)
            nc.sync.dma_start(const_sb[:], const_dram[:])
            cbf_sb = consts.tile([128, 2 * K], F8, tag="cbf")
            nc.scalar.dma_start(cbf_sb[:], cbf_dram[:])
            o = 0
            iota_sb = const_sb[:, o:o + C]; o += C
            lab_sb = const_sb[:, o:o + tiles]; o += tiles
            w_sb = const_sb[:, o:o + tiles]; o += tiles
            cnorm_sb = const_sb[:, o:o + K]; o += K

            # x chunks: issued upfront from one sequencer (interleaving
            # two DGE descriptor streams measurably slows the HBM stream)
            xts = []
            for c in range(nchunk):
                xt = xin.tile([128, 2, CHUNK, 128], F8, tag=f"x{c}")
                nc.gpsimd.dma_start(
                    xt[:], x_dram[:, :, c * CHUNK:(c + 1) * CHUNK, :]
                )
                xts.append(xt)

            psum_segA = psseg.tile([NM, OHC], F32, tag="segA")
            psum_segB = psseg.tile([NM, OHC], F32, tag="segB")

            # one-hot blocks + vals const col: depend only on consts, the
            # scheduler runs them during the DMA fill
            ohs, valss = [], []
            for b, gb in enumerate(batches):
                t0 = starts[b]
                oh = bigp.tile([128, gb, OHC], BF16, tag=f"oh{b}")
                nc.vector.tensor_tensor(
                    oh[:, :, C:OHC], _b0(iota_sb, gb, "outer"),
                    _b0(lab_sb[:, t0:t0 + gb], C, "inner"), ALU.is_equal,
                )
                nc.vector.tensor_tensor(
                    oh[:, :, 0:C], oh[:, :, C:OHC],
                    _b0(w_sb[:, t0:t0 + gb], C, "inner"), ALU.mult,
                )
                vals = bigp.tile([128, gb, NM], BF16, tag=f"vals{b}")
                nc.vector.memset(vals[:, :, 11:12], 1.0)
                ohs.append(oh)
                valss.append(vals)

            def stage_a(b):
                gb = batches[b]
                t0 = starts[b]
                psd = psd_pool.tile([128, gb, K], F32, tag=f"psd{b}")
                for g in range(gb):
                    t = t0 + g
                    xt = xts[t // CHUNK]
                    r = t % CHUNK
                    nc.tensor.matmul(
                        psd[:, g, :], xt[:, 0, r, :], cbf_sb[:, 0:K],
                        start=True, stop=False,
                    )
                    nc.tensor.matmul(
                        psd[:, g, :], xt[:, 1, r, :], cbf_sb[:, K:2 * K],
                        start=False, stop=True,
                    )
                vals = valss[b]
                # d2 = psum + (1 + |c|^2)
                t_d2 = small.tile([128, gb, K], BF16, tag="t_d2")
                nc.vector.tensor_tensor(
                    t_d2[:], psd[:], _b0(cnorm_sb, gb, "outer"), ALU.add,
                )
                # ACT block: dist = exp(0.5*ln(d2)); eu = exp(-dist)
                lnt = small.tile([128, gb, K], F32, tag="lnt")
                nc.scalar.activation(lnt[:], t_d2[:], ACTF.Ln)
                nc.scalar.activation(vals[:, :, 0:K], lnt[:], ACTF.Exp,
                                     scale=0.5)
                eu = small.tile([128, gb, K], BF16, tag="eu")
                nc.scalar.activation(eu[:], vals[:, :, 0:K], ACTF.Exp,
                                     scale=-1.0)
                return eu

            def stage_b(b, eu):
                gb = batches[b]
                vals = valss[b]
                dist = vals[:, :, 0:K]
                m1 = stat.tile([128, gb], BF16, tag="m1")
                nc.vector.tensor_reduce(m1[:], dist, axis=AX.X, op=ALU.min)
                maskB = small.tile([128, gb, K], BF16, tag="maskB")
                nc.vector.tensor_tensor(maskB[:], dist, _b0(m1[:], K),
                                        ALU.is_equal)
                dmask = small.tile([128, gb, K], BF16, tag="dmask")
                nc.vector.tensor_scalar(dmask[:], maskB[:], BIG, None,
                                        ALU.mult)
                nc.vector.tensor_tensor(dmask[:], dmask[:], dist, ALU.add)
                m2 = stat.tile([128, gb], BF16, tag="m2")
                nc.vector.tensor_reduce(m2[:], dmask[:], axis=AX.X, op=ALU.min)
                delta = stat.tile([128, gb], BF16, tag="delta")
                nc.vector.tensor_tensor(delta[:], m2[:], m1[:], ALU.subtract)
                # softmax-weighted dist: wd = sum(eu*d)/sum(eu)
                s = stat.tile([128, gb], F32, tag="s")
                nc.vector.tensor_reduce(s[:], eu[:], axis=AX.X, op=ALU.add)
                prod = small.tile([128, gb, K], BF16, tag="prod")
                nc.vector.tensor_tensor(prod[:], eu[:], dist, ALU.mult)
                spd = stat.tile([128, gb], F32, tag="spd")
                nc.vector.tensor_reduce(spd[:], prod[:], axis=AX.X, op=ALU.add)
                rs = stat.tile([128, gb], F32, tag="rs")
                nc.vector.reciprocal(rs[:], s[:])
                wd = stat.tile([128, gb], F32, tag="wd")
                nc.vector.tensor_tensor(wd[:], spd[:], rs[:], ALU.mult)
                # vals[:, :, 10] = wd^2 (col 11 pre-set to 1)
                wd3 = wd[:].rearrange("p (g o) -> p g o", o=1)
                nc.vector.tensor_tensor(vals[:, :, 10:11], wd3, wd3, ALU.mult)
                # viol_j = relu(wd + margin - mo_j), mo_j = min_{k!=j} d_k
                #        = m1 + (m2-m1)*[d_j==m1]
                # hng_j = (wd - m1) - (m2-m1)*[d_j==m1]; viol = max(hng+M, 0)
                wdm1 = stat.tile([128, gb], F32, tag="wdm1")
                nc.vector.tensor_tensor(wdm1[:], wd[:], m1[:], ALU.subtract)
                hng = small.tile([128, gb, K], BF16, tag="hng")
                nc.vector.tensor_tensor(hng[:], maskB[:], _b0(delta[:], K),
                                        ALU.mult)
                nc.vector.tensor_tensor(hng[:], _b0(wdm1[:], K), hng[:],
                                        ALU.subtract)
                nc.vector.tensor_scalar(vals[:, :, K:2 * K], hng[:], MARGIN,
                                        0.0, ALU.add, ALU.max)

            for b in range(len(batches)):
                eu = stage_a(b)
                stage_b(b, eu)

            nbat = len(batches)
            # seg bank A: batches 0..nbat-2, sim-floored past the x stream so
            # the scheduler never wedges it between psd matmul groups
            with tc.tile_wait_until(SEG_FLOOR_MS):
                first = True
                for b in range(nbat - 1):
                    vals, oh, gb = valss[b], ohs[b], batches[b]
                    for g in range(gb):
                        nc.tensor.matmul(
                            psum_segA[:], vals[:, g, :], oh[:, g, :],
                            start=first,
                            stop=(b == nbat - 2 and g == gb - 1),
                        )
                        first = False
            # seg bank B: the small last batch (short post-stream tail)
            b = nbat - 1
            vals, oh, gb = valss[b], ohs[b], batches[b]
            for g in range(gb):
                nc.tensor.matmul(
                    psum_segB[:], vals[:, g, :], oh[:, g, :],
                    start=(g == 0), stop=(g == gb - 1),
                )

            segb_sb = consts.tile([NM, OHC], F32, tag="segb_sb")
            nc.vector.tensor_copy(segb_sb[:], psum_segB[:])
            seg_sb = consts.tile([NM, OHC], F32, tag="seg_sb")
            nc.vector.tensor_tensor(seg_sb[:], psum_segA[:], segb_sb[:],
                                    ALU.add)
            nc.sync.dma_start(out_dram[:], seg_sb[:])

    nc.compile()
    return nc


def _host_prep(feat, labels, label2, centers, tiles=TILES,
               n_cores=NCORES):
    """Pad + shard + pre-transpose + bf16-cast to per-core arrays."""
    import ml_dtypes

    rpc = tiles * 128
    bpad = rpc * n_cores
    b = feat.shape[0]

    feat = np.asarray(feat, dtype=np.float32)
    labels = np.asarray(labels)
    label2 = np.asarray(label2)
    centers = np.asarray(centers, dtype=np.float32)

    lab_f = np.full(bpad, float(C), dtype=np.float32)
    lab_f[:b] = labels.astype(np.float32)
    w_f = np.zeros(bpad, dtype=np.float32)
    w_f[:b] = (label2 == 1).astype(np.float32)
    xpad = np.zeros((bpad, D), dtype=np.float32)
    xpad[:b] = feat

    # constants
    ctilT = (-2.0 / XSCALE * centers.T).astype(np.float32)  # [256, 5]
    cbf = np.ascontiguousarray(
        np.concatenate([ctilT[0:128], ctilT[128:256]], axis=1)
    ).astype(ml_dtypes.float8_e4m3)                        # [128, 10]
    cnorm1 = 1.0 + (centers * centers).sum(axis=1).astype(np.float32)  # [5]
    iota = np.tile(np.arange(C, dtype=np.float32)[None, :], (128, 1))
    cn_rep = np.tile(cnorm1[None, :], (128, 1))

    in_maps = []
    for i in range(n_cores):
        sl = slice(i * rpc, (i + 1) * rpc)
        # XT layout [dpart, dchunk, tile, row]:
        #   x[dp, c, t, r] = feat[t*128 + r, c*128 + dp]
        xi = np.ascontiguousarray(
            (xpad[sl] * XSCALE).reshape(tiles, 128, 2, 128)
            .transpose(3, 2, 0, 1)
        ).astype(ml_dtypes.float8_e4m3)
        li = np.ascontiguousarray(lab_f[sl].reshape(tiles, 128).T)
        wi = np.ascontiguousarray(w_f[sl].reshape(tiles, 128).T)
        const = np.concatenate([iota, li, wi, cn_rep], axis=1)
        in_maps.append(
            {"x": xi, "const": np.ascontiguousarray(const), "cbf": cbf}
        )
    return in_maps


def _host_final(seg):
    """Final stage on the all-reduced [12, 32] stats (exact reference math).
    Cols 0:16 are w-weighted sums, cols 16:32 unweighted."""
    seg = seg.astype(np.float64)
    wblk = seg[:, 0:C]
    pblk = seg[:, C:OHC]
    sum_dist = wblk[0:K].T         # [C, K]
    sum_violj = wblk[K:2 * K].T    # [C, K]
    sum_wd2 = wblk[10]             # [C]
    cnt = wblk[11]                 # [C]
    present = pblk[11]             # [C]
    safe = np.maximum(cnt, 1.0)
    closest = np.argmin(sum_dist / safe[:, None], axis=1)
    sum_viol = sum_violj[np.arange(C), closest]
    has = (cnt > 0).astype(np.float64)
    per_class = (sum_wd2 + sum_viol) / safe * has
    n_unique = max(float((present > 0).sum()), 1.0)
    return np.float32(per_class.sum() / n_unique)


_NC_CACHE = {}


def kernel(feat_normed, labels, label2, num_classes, centers, _trace=False):
    if "nc" not in _NC_CACHE:
        _NC_CACHE["nc"] = build_nc()
    nc = _NC_CACHE["nc"]
    in_maps = _host_prep(feat_normed, labels, label2, centers)
    res = run_bass_kernel_spmd(
        nc, in_maps, core_ids=list(range(NCORES)), trace=_trace
    )
    seg = np.zeros((NM, OHC), dtype=np.float64)
    for r in res.results:
        seg += np.asarray(r["out"], dtype=np.float64)
    if _trace:
        kernel.last_result = res
    return np.asarray(_host_final(seg), dtype=np.float32)
